# revision 3
# baseline (speedup 1.0000x reference)
"""Trainium2 Bass kernel for a 4-head GAT layer (N=4096, D=256, O=64, H=4).

Math (reference):
    feat[h] = X @ W[h]                                  [N, O]
    s[h,i] = feat[h,i] @ a_src[h],  t[h,j] = feat[h,j] @ a_dst[h]
    score[h,i,j] = leaky_relu(s_i + t_j, 0.2), masked by A>0, softmax over j
    out[i, h*O+o] = sum_j attn[h,i,j] feat[h,j,o] + b[h,o]

Key factorization used on-device (avoids 67M-element exp/leaky passes):
    exp(leaky_relu(x)) = max(e^x, e^{0.2x}); with x = s_i + t_j both branches
    factor:  e^x = e^{s_i} e^{t_j},  e^{0.2x} = e^{0.2 s_i} e^{0.2 t_j}.
    With M2 = A * [x >= 0] (one fused DVE scalar_tensor_tensor op per tile)
    and M1 = A - M2:
      numer = e^{0.2 s} * (M1 @ (q*feat)) + e^{s} * (M2 @ (v*feat))
            = e^{0.2 s} * [ (A@(q*feat) - M2@(q*feat)) + e^{0.8 s} * (M2@(v*feat)) ]
    where v = e^t, q = e^{0.2 t}.  The common e^{0.2 s} cancels in the
    softmax ratio, so only w = e^{0.8 s} is needed.  Row sums come from an
    appended ones-column in the rhs ("q*1 = q", "v*1 = v").

Sharding: rows (destination nodes) are split 512/core across 8 cores; the
source-side features (all N columns) are recomputed on every core (cheap).
No collectives.  b is always zero in setup_inputs, but is added on the host
for faithfulness.
"""

import os
from contextlib import ExitStack

import numpy as np
import ml_dtypes

import concourse.bass as bass
import concourse.tile as tile
import concourse.mybir as mybir
from concourse import bacc
from concourse.bass_utils import run_bass_kernel_spmd

P = 128
IN_DIM = 256
OUT_DIM = 64
HEADS = 4
N_TOTAL = 4096
N_CORES = 8
ROWS = N_TOTAL // N_CORES  # 512

F32 = mybir.dt.float32
F16 = mybir.dt.float16

AL = mybir.AluOpType
AF = mybir.ActivationFunctionType


def build_program(n_total=N_TOTAL, rows=ROWS, num_devices=N_CORES):
    """Build the per-core SPMD program.  All cores run this same program;
    per-core data arrives via the input map."""
    ntiles = n_total // P   # source-node tiles (j), also feat tiles
    nib = rows // P         # destination row blocks per core
    njt = ntiles

    nc = bacc.Bacc("TRN2", target_bir_lowering=False, debug=False,
                   num_devices=num_devices)

    # ---- DRAM I/O (names are the in_map keys) ----
    XT = nc.dram_tensor("XT", [IN_DIM, n_total], F16, kind="ExternalInput")
    XTOWN = nc.dram_tensor("XTOWN", [IN_DIM, rows], F16, kind="ExternalInput")
    # W8: per-head W packed [d, h*64+o] (256 cols) | w_dst per head (4 cols)
    W8 = nc.dram_tensor("W8", [IN_DIM, 260], F16, kind="ExternalInput")
    # w_src per head (4 cols) for s_own
    W4 = nc.dram_tensor("W4", [IN_DIM, 4], F16, kind="ExternalInput")
    # w_src replicated across 128 cols per head, for the s-broadcast matmul
    WSRCB = nc.dram_tensor("WSRCB", [IN_DIM, 4 * P], F16, kind="ExternalInput")
    # A[i0:i0+rows, :].T  as bf16 (0.0/1.0)
    AT = nc.dram_tensor("AT", [n_total, rows], F16, kind="ExternalInput")
    OUT = nc.dram_tensor("OUT", [rows, HEADS * OUT_DIM], F32,
                         kind="ExternalOutput")

    with tile.TileContext(nc) as tc, ExitStack() as ctx:
        big = ctx.enter_context(tc.tile_pool(name="big", bufs=1))

        # ---- Phase 0: load everything ----
        xt_sb = big.tile([P, 2 * n_total], F16, tag="xt")
        for d in range(2):
            nc.sync.dma_start(xt_sb[:, d * n_total:(d + 1) * n_total],
                              XT[d * P:(d + 1) * P, :])
        xtown_sb = big.tile([P, 2 * rows], F16, tag="xtown")
        for d in range(2):
            nc.sync.dma_start(xtown_sb[:, d * rows:(d + 1) * rows],
                              XTOWN[d * P:(d + 1) * P, :])
        w8_sb = big.tile([P, 2 * 260], F16, tag="w8")
        for d in range(2):
            nc.sync.dma_start(w8_sb[:, d * 260:(d + 1) * 260],
                              W8[d * P:(d + 1) * P, :])
        w4_sb = big.tile([P, 2 * 4], F16, tag="w4")
        for d in range(2):
            nc.sync.dma_start(w4_sb[:, d * 4:(d + 1) * 4],
                              W4[d * P:(d + 1) * P, :])
        wsrcb_sb = big.tile([P, 2 * 4 * P], F16, tag="wsrcb")
        for d in range(2):
            nc.sync.dma_start(wsrcb_sb[:, d * 4 * P:(d + 1) * 4 * P],
                              WSRCB[d * P:(d + 1) * P, :])
        at_sb = big.tile([P, njt * rows], F16, tag="at")
        for jt in range(njt):
            nc.sync.dma_start(at_sb[:, jt * rows:(jt + 1) * rows],
                              AT[jt * P:(jt + 1) * P, :])

        # ---- Phase 1: feat (+ t) matmuls ----
        # fet per n-tile: [feat_h(64)|1]*4  (260) | t_h (4)  => 264 cols
        fet = big.tile([P, ntiles * 264], F16, tag="fet")
        fet3 = fet[:].rearrange("p (n c) -> p n c", c=264)
        with tc.tile_pool(name="pfeat", bufs=2, space=bass.MemorySpace.PSUM) as pf:
            for nt in range(ntiles):
                ps = pf.tile([P, 264], F32, tag="ps")
                for d in range(2):
                    nc.tensor.matmul(
                        ps[:, 0:260],
                        xt_sb[:, d * n_total + nt * P: d * n_total + (nt + 1) * P],
                        w8_sb[:, d * 260:(d + 1) * 260],
                        start=(d == 0), stop=(d == 1))
                # feat -> strided 65-groups (leave col 64 of each group for ones)
                nc.scalar.activation(
                    fet3[:, nt, 0:260].rearrange("p (g c) -> p g c", c=65)[:, :, 0:64],
                    ps[:, 0:256].rearrange("p (g c) -> p g c", c=64),
                    AF.Copy)
                # t columns
                nc.scalar.activation(fet3[:, nt, 260:264], ps[:, 256:260], AF.Copy)
                # ones columns
                nc.vector.memset(
                    fet3[:, nt, 0:260].rearrange("p (g c) -> p g c", c=65)[:, :, 64:65],
                    1.0)

        # ---- Phase 2: small vectors ----
        # vq per n-tile: [v_h (4) | q_h (4)]
        vq = big.tile([P, ntiles * 8], F16, tag="vq")
        vq3 = vq[:].rearrange("p (n c) -> p n c", c=8)
        nc.scalar.activation(vq3[:, :, 0:4], fet3[:, :, 260:264], AF.Exp)
        nc.scalar.activation(vq3[:, :, 4:8], fet3[:, :, 260:264], AF.Exp, scale=0.2)
        negt = big.tile([P, ntiles * 4], F32, tag="negt")
        nc.vector.tensor_scalar_mul(negt[:], fet3[:, :, 260:264], -1.0)

        # s_own / w_cat
        s_own = big.tile([P, nib * 4], F32, tag="s_own")
        w_cat = big.tile([P, nib * 4], F32, tag="w_cat")
        with tc.tile_pool(name="pso", bufs=1, space=bass.MemorySpace.PSUM) as pso:
            ps = pso.tile([P, nib * 4], F32, tag="ps_so")
            for ib in range(nib):
                for d in range(2):
                    nc.tensor.matmul(
                        ps[:, ib * 4:(ib + 1) * 4],
                        xtown_sb[:, d * rows + ib * P: d * rows + (ib + 1) * P],
                        w4_sb[:, d * 4:(d + 1) * 4],
                        start=(d == 0), stop=(d == 1))
            nc.vector.tensor_copy(s_own[:], ps[:])
        nc.scalar.activation(w_cat[:], s_own[:], AF.Exp, scale=0.8)

        # s broadcast tiles: sbc[ib, h] = [128, 128] tile, every partition row
        # holds s_src[h, i] for i in this row-block.
        sbc = big.tile([P, nib * 4 * P], F16, tag="sbc")
        with tc.tile_pool(name="psb", bufs=2, space=bass.MemorySpace.PSUM) as psb:
            for ib in range(nib):
                ps = psb.tile([P, 4 * P], F32, tag="ps_sb")
                for h in range(HEADS):
                    for d in range(2):
                        nc.tensor.matmul(
                            ps[:, h * P:(h + 1) * P],
                            wsrcb_sb[:, d * 4 * P + h * P: d * 4 * P + (h + 1) * P],
                            xtown_sb[:, d * rows + ib * P: d * rows + (ib + 1) * P],
                            start=(d == 0), stop=(d == 1))
                nc.vector.tensor_copy(sbc[:, ib * 4 * P:(ib + 1) * 4 * P], ps[:])

        # ---- Phase 3: vfqf rhs panels ----
        # per n-tile: [vf(64)|v | qf(64)|q] per head = 130 cols/head, 520 total
        vfqf = big.tile([P, ntiles * 520], F16, tag="vfqf")
        for nt in range(ntiles):
            vfqf3 = vfqf[:, nt * 520:(nt + 1) * 520].rearrange(
                "p (g c) -> p g c", c=130)
            fe65 = fet3[:, nt, 0:260].rearrange("p (g c) -> p g c", c=65)
            v_b = vq3[:, nt, 0:4].unsqueeze(2).to_broadcast([P, 4, 65])
            q_b = vq3[:, nt, 4:8].unsqueeze(2).to_broadcast([P, 4, 65])
            nc.vector.tensor_tensor(vfqf3[:, :, 0:65], fe65, v_b, AL.mult)
            nc.vector.tensor_tensor(vfqf3[:, :, 65:130], fe65, q_b, AL.mult)

        # ---- Phase 4: masked attention matmuls + epilogue ----
        out_sb_pool = ctx.enter_context(tc.tile_pool(name="osb", bufs=2))
        m2_pool = ctx.enter_context(tc.tile_pool(name="m2", bufs=8))
        e_pool = ctx.enter_context(tc.tile_pool(name="epi", bufs=4))
        with tc.tile_pool(name="pA", bufs=2, space=bass.MemorySpace.PSUM) as pA, \
             tc.tile_pool(name="pB", bufs=4, space=bass.MemorySpace.PSUM) as pB:
            for ib in range(nib):
                pa = pA.tile([P, 260], F32, tag="pa")
                pb = []
                for h in range(HEADS):
                    pbh = pB.tile([P, 130], F32, tag="pb")
                    pb.append(pbh)
                for jt in range(njt):
                    a_ap = at_sb[:, jt * rows + ib * P: jt * rows + (ib + 1) * P]
                    vf_nt = vfqf[:, jt * 520:(jt + 1) * 520]
                    # A @ qf panel (all heads at once, strided rhs)
                    nc.tensor.matmul(
                        pa[:],
                        a_ap,
                        vf_nt.rearrange("p (g c) -> p g c", c=130)[:, :, 65:130],
                        start=(jt == 0), stop=(jt == njt - 1))
                    for h in range(HEADS):
                        m2t = m2_pool.tile([P, P], F16, tag="m2t")
                        nc.vector.scalar_tensor_tensor(
                            m2t[:],
                            sbc[:, (ib * 4 + h) * P: (ib * 4 + h + 1) * P],
                            negt[:, jt * 4 + h: jt * 4 + h + 1],
                            a_ap,
                            AL.is_ge, AL.mult)
                        nc.tensor.matmul(
                            pb[h][:],
                            m2t[:],
                            vf_nt[:, h * 130:(h + 1) * 130],
                            start=(jt == 0), stop=(jt == njt - 1))
                # epilogue for this row-block
                ca = e_pool.tile([P, 260], F32, tag="ca")
                nc.scalar.activation(ca[:], pa[:], AF.Copy)
                out_sb = out_sb_pool.tile([P, HEADS * OUT_DIM], F32, tag="outsb")
                for h in range(HEADS):
                    dh = e_pool.tile([P, 65], F32, tag="dh")
                    nc.vector.tensor_sub(dh[:], ca[:, h * 65:(h + 1) * 65],
                                         pb[h][:, 65:130])
                    zh = e_pool.tile([P, 65], F32, tag="zh")
                    nc.vector.scalar_tensor_tensor(
                        zh[:], pb[h][:, 0:65],
                        w_cat[:, ib * 4 + h: ib * 4 + h + 1],
                        dh[:], AL.mult, AL.add)
                    rc = e_pool.tile([P, 1], F32, tag="rc")
                    nc.vector.reciprocal(rc[:], zh[:, 64:65])
                    nc.vector.tensor_scalar_mul(
                        out_sb[:, h * OUT_DIM:(h + 1) * OUT_DIM],
                        zh[:, 0:OUT_DIM], rc[:])
                nc.sync.dma_start(OUT[ib * P:(ib + 1) * P, :], out_sb[:])

    nc.compile()
    return nc


def prep_inputs(X, A, W, a, n_total=N_TOTAL, rows=ROWS, n_cores=N_CORES):
    """Host-side sharding / layout prep.  Returns list of per-core in_maps."""
    f16 = np.float16
    X = np.asarray(X, np.float32)
    A = np.asarray(A)
    W = np.asarray(W, np.float32)
    a = np.asarray(a, np.float32)

    XT = np.ascontiguousarray(X.T).astype(f16)                       # [256, N]
    # W8: [d, h*64+o] | w_dst[h]
    Wcat = np.ascontiguousarray(W.transpose(1, 0, 2).reshape(IN_DIM, HEADS * OUT_DIM))
    a_src, a_dst = a[:, :OUT_DIM], a[:, OUT_DIM:]
    w_src = np.einsum('hdo,ho->hd', W, a_src).astype(np.float32)    # [4, 256]
    w_dst = np.einsum('hdo,ho->hd', W, a_dst).astype(np.float32)
    W8 = np.concatenate([Wcat, w_dst.T], axis=1).astype(f16)         # [256, 260]
    W4 = np.ascontiguousarray(w_src.T).astype(f16)                   # [256, 4]
    WSRCB = np.repeat(w_src.T[:, :, None], P, axis=2).reshape(IN_DIM, HEADS * P)
    WSRCB = np.ascontiguousarray(WSRCB).astype(f16)

    Af = (A > 0).astype(np.float32)
    in_maps = []
    for c in range(n_cores):
        i0 = c * rows
        at = np.ascontiguousarray(Af[i0:i0 + rows, :].T).astype(f16)  # [N, rows]
        xtown = np.ascontiguousarray(X[i0:i0 + rows, :].T).astype(f16)
        in_maps.append({
            "XT": XT, "XTOWN": xtown, "W8": W8, "W4": W4,
            "WSRCB": WSRCB, "AT": at,
        })
    return in_maps


_CACHED_NC = None


def _get_nc():
    global _CACHED_NC
    if _CACHED_NC is None:
        _CACHED_NC = build_program()
    return _CACHED_NC


def kernel(X, A, W, a, b, _trace=False, _trace_kwargs=None):
    nc = _get_nc()
    in_maps = prep_inputs(X, A, W, a)
    kw = {}
    if _trace:
        kw["trace"] = True
        if _trace_kwargs:
            kw.update(_trace_kwargs)
    res = run_bass_kernel_spmd(nc, in_maps, core_ids=list(range(N_CORES)), **kw)
    out = np.concatenate([r["OUT"] for r in res.results], axis=0)
    out = out + np.asarray(b, np.float32).reshape(1, HEADS * OUT_DIM)
    if _trace:
        return out.astype(np.float32), res
    return out.astype(np.float32)


# revision 7
# speedup vs baseline: 1.4671x; 1.4671x over previous
"""Trainium2 Bass kernel for a 4-head GAT layer (N=4096, D=256, O=64, H=4).

Math (reference):
    feat[h] = X @ W[h]                                  [N, O]
    s[h,i] = feat[h,i] @ a_src[h],  t[h,j] = feat[h,j] @ a_dst[h]
    score[h,i,j] = leaky_relu(s_i + t_j, 0.2), masked by A>0, softmax over j
    out[i, h*O+o] = sum_j attn[h,i,j] feat[h,j,o] + b[h,o]

Key factorization used on-device (avoids 67M-element exp/leaky passes):
    exp(leaky_relu(x)) = max(e^x, e^{0.2x}); with x = s_i + t_j both branches
    factor.  With M2 = A * [x >= 0] and M1 = A - M2:
      numer = e^{0.2 s} * [ (A@(q*f) - M2@(q*f)) + e^{0.8 s} * (M2@(v*f)) ]
    where v = e^t, q = e^{0.2 t}; the common e^{0.2 s} cancels in the softmax
    ratio.  Row sums come from an appended ones-column in the rhs panels.

    Masks are built with DVE fast modes: tensor_scalar is_ge (4x mode) for
    m = [s_i + t_j >= 0] batched [128 x 512], then tensor_tensor mult by the
    A tile (2x mode).  Heads are processed in two passes of 2 so the
    per-row-block PSUM accumulators (A-branch + M2-branch) fit in 8 banks.

Sharding: destination rows are split 512/core across 8 cores; source-side
features (all N) are recomputed per core (cheap).  No collectives.
b is always zero in setup_inputs but is added on the host anyway.
"""

from contextlib import ExitStack

import numpy as np

import concourse.bass as bass
import concourse.tile as tile
import concourse.mybir as mybir
from concourse import bacc
from concourse.bass_utils import run_bass_kernel_spmd

P = 128
IN_DIM = 256
OUT_DIM = 64
HEADS = 4
N_TOTAL = 4096
N_CORES = 8
ROWS = N_TOTAL // N_CORES  # 512

F32 = mybir.dt.float32
F16 = mybir.dt.float16

AL = mybir.AluOpType
AF = mybir.ActivationFunctionType

GRP = 66          # [feat(64) | 1 | 1] per head in the fe panel (even => 4B aligned)
FET_C = 4 * GRP + 4   # 268 cols per n-tile in fet
VQ_C = 2 * GRP    # 132 cols per head in the vfqf panel
PAN_C = 4 * VQ_C  # 528 cols per n-tile in vfqf


def build_program(n_total=N_TOTAL, rows=ROWS, num_devices=N_CORES):
    """Build the per-core SPMD program (same program on all cores; per-core
    data arrives via the input map)."""
    ntiles = n_total // P   # source-node tiles (j)
    nib = rows // P         # destination row blocks per core
    njt = ntiles

    nc = bacc.Bacc("TRN2", target_bir_lowering=False, debug=False,
                   num_devices=num_devices)

    XT = nc.dram_tensor("XT", [IN_DIM, n_total], F16, kind="ExternalInput")
    XTOWN = nc.dram_tensor("XTOWN", [IN_DIM, rows], F16, kind="ExternalInput")
    W8 = nc.dram_tensor("W8", [IN_DIM, 260], F16, kind="ExternalInput")
    W4 = nc.dram_tensor("W4", [IN_DIM, 4], F16, kind="ExternalInput")
    WSRCB = nc.dram_tensor("WSRCB", [IN_DIM, 4 * P], F16, kind="ExternalInput")
    AT = nc.dram_tensor("AT", [n_total, rows], F16, kind="ExternalInput")
    OUT = nc.dram_tensor("OUT", [rows, HEADS * OUT_DIM], F32,
                         kind="ExternalOutput")

    with tile.TileContext(nc) as tc, ExitStack() as ctx:
        big = ctx.enter_context(tc.tile_pool(name="big", bufs=1))

        # ---- Phase 0: load everything ----
        xt_sb = big.tile([P, 2 * n_total], F16, tag="xt")
        for d in range(2):
            nc.sync.dma_start(xt_sb[:, d * n_total:(d + 1) * n_total],
                              XT[d * P:(d + 1) * P, :])
        xtown_sb = big.tile([P, 2 * rows], F16, tag="xtown")
        for d in range(2):
            nc.sync.dma_start(xtown_sb[:, d * rows:(d + 1) * rows],
                              XTOWN[d * P:(d + 1) * P, :])
        w8_sb = big.tile([P, 2 * 260], F16, tag="w8")
        for d in range(2):
            nc.sync.dma_start(w8_sb[:, d * 260:(d + 1) * 260],
                              W8[d * P:(d + 1) * P, :])
        w4_sb = big.tile([P, 2 * 4], F16, tag="w4")
        for d in range(2):
            nc.sync.dma_start(w4_sb[:, d * 4:(d + 1) * 4],
                              W4[d * P:(d + 1) * P, :])
        wsrcb_sb = big.tile([P, 2 * 4 * P], F16, tag="wsrcb")
        for d in range(2):
            nc.sync.dma_start(wsrcb_sb[:, d * 4 * P:(d + 1) * 4 * P],
                              WSRCB[d * P:(d + 1) * P, :])
        at_sb = big.tile([P, njt * rows], F16, tag="at")
        for jt in range(njt):
            nc.sync.dma_start(at_sb[:, jt * rows:(jt + 1) * rows],
                              AT[jt * P:(jt + 1) * P, :])

        # ---- Phase 1: feat (+ t) matmuls ----
        fet = big.tile([P, ntiles * FET_C], F16, tag="fet")
        fet3 = fet[:].rearrange("p (n c) -> p n c", c=FET_C)
        with tc.tile_pool(name="pfeat", bufs=2, space=bass.MemorySpace.PSUM) as pf:
            for nt in range(ntiles):
                ps = pf.tile([P, 264], F32, tag="ps")
                for d in range(2):
                    nc.tensor.matmul(
                        ps[:, 0:260],
                        xt_sb[:, d * n_total + nt * P: d * n_total + (nt + 1) * P],
                        w8_sb[:, d * 260:(d + 1) * 260],
                        start=(d == 0), stop=(d == 1))
                fe_g = fet3[:, nt, 0:4 * GRP].rearrange("p (g c) -> p g c", c=GRP)
                nc.scalar.activation(
                    fe_g[:, :, 0:64],
                    ps[:, 0:256].rearrange("p (g c) -> p g c", c=64),
                    AF.Copy)
                nc.scalar.activation(fet3[:, nt, 4 * GRP:FET_C],
                                     ps[:, 256:260], AF.Copy)
                # ones column (64) + neutral pad (65)
                nc.vector.memset(fe_g[:, :, 64:66], 1.0)

        # ---- Phase 2: small vectors ----
        vq = big.tile([P, ntiles * 8], F32, tag="vq")
        vq3 = vq[:].rearrange("p (n c) -> p n c", c=8)
        nc.scalar.activation(vq3[:, :, 0:4], fet3[:, :, 4 * GRP:FET_C], AF.Exp)
        nc.scalar.activation(vq3[:, :, 4:8], fet3[:, :, 4 * GRP:FET_C], AF.Exp,
                             scale=0.2)
        negt = big.tile([P, ntiles * 4], F32, tag="negt")
        nc.vector.tensor_scalar_mul(negt[:], fet3[:, :, 4 * GRP:FET_C], -1.0)

        s_own = big.tile([P, nib * 4], F32, tag="s_own")
        w_cat = big.tile([P, nib * 4], F32, tag="w_cat")
        with tc.tile_pool(name="pso", bufs=1, space=bass.MemorySpace.PSUM) as pso:
            ps = pso.tile([P, nib * 4], F32, tag="ps_so")
            for ib in range(nib):
                for d in range(2):
                    nc.tensor.matmul(
                        ps[:, ib * 4:(ib + 1) * 4],
                        xtown_sb[:, d * rows + ib * P: d * rows + (ib + 1) * P],
                        w4_sb[:, d * 4:(d + 1) * 4],
                        start=(d == 0), stop=(d == 1))
            nc.vector.tensor_copy(s_own[:], ps[:])
        nc.scalar.activation(w_cat[:], s_own[:], AF.Exp, scale=0.8)

        # s broadcast rows, grouped per head: sbc[:, h*rows + i] = s_src[h, i]
        sbc = big.tile([P, 4 * rows], F16, tag="sbc")
        with tc.tile_pool(name="psb", bufs=2, space=bass.MemorySpace.PSUM) as psb:
            for ib in range(nib):
                ps = psb.tile([P, 4 * P], F32, tag="ps_sb")
                for h in range(HEADS):
                    for d in range(2):
                        nc.tensor.matmul(
                            ps[:, h * P:(h + 1) * P],
                            wsrcb_sb[:, d * 4 * P + h * P: d * 4 * P + (h + 1) * P],
                            xtown_sb[:, d * rows + ib * P: d * rows + (ib + 1) * P],
                            start=(d == 0), stop=(d == 1))
                for h in range(HEADS):
                    nc.vector.tensor_copy(
                        sbc[:, h * rows + ib * P: h * rows + (ib + 1) * P],
                        ps[:, h * P:(h + 1) * P])

        # ---- Phase 3: vfqf rhs panels ----
        # per n-tile, per head: [vf(64)|v|pad | qf(64)|q|pad]  (132 cols)
        vfqf = big.tile([P, ntiles * PAN_C], F16, tag="vfqf")
        for nt in range(ntiles):
            fe_g = fet3[:, nt, 0:4 * GRP].rearrange("p (g c) -> p g c", c=GRP)
            pan = vfqf[:, nt * PAN_C:(nt + 1) * PAN_C]
            for h in range(HEADS):
                nc.vector.tensor_scalar_mul(
                    pan[:, h * VQ_C: h * VQ_C + GRP],
                    fe_g[:, h, :], vq3[:, nt, h:h + 1])
                nc.vector.tensor_scalar_mul(
                    pan[:, h * VQ_C + GRP: (h + 1) * VQ_C],
                    fe_g[:, h, :], vq3[:, nt, 4 + h:5 + h])

        # ---- Phase 4a: A-branch pre-pass (all heads at once) ----
        # pa_all[ib] accumulates A @ [qf_h|q_h for h in 0..4]  (N=260).
        m_pool = ctx.enter_context(tc.tile_pool(name="m", bufs=3))
        m2_pool = ctx.enter_context(tc.tile_pool(name="m2", bufs=4))
        out_sb_pool = ctx.enter_context(tc.tile_pool(name="osb", bufs=2))
        e_pool = ctx.enter_context(tc.tile_pool(name="epi", bufs=6))
        ca_all = []
        for ib in range(nib):
            ca_ib = big.tile([P, 260], F32, tag=f"ca{ib}")
            ca_all.append(ca_ib)
        with tc.tile_pool(name="pA", bufs=4, space=bass.MemorySpace.PSUM) as pA:
            pa = []
            for ib in range(nib):
                pa_ib = pA.tile([P, 260], F32, tag="pa")
                pa.append(pa_ib)
            for jt in range(njt):
                pan = vfqf[:, jt * PAN_C:(jt + 1) * PAN_C]
                qf_all = pan[:].rearrange("p (g c) -> p g c", c=VQ_C)[:, :, GRP:GRP + 65]
                for ib in range(nib):
                    nc.tensor.matmul(
                        pa[ib][:],
                        at_sb[:, jt * rows + ib * P: jt * rows + (ib + 1) * P],
                        qf_all,
                        start=(jt == 0), stop=(jt == njt - 1))
            for ib in range(nib):
                nc.scalar.activation(ca_all[ib][:], pa[ib][:], AF.Copy)

        # ---- Phase 4b: M2-branch, two passes of 2 heads ----
        out_sbs = []
        for ib in range(nib):
            osb = out_sb_pool.tile([P, HEADS * OUT_DIM], F32, tag="outsb")
            out_sbs.append(osb)
        with tc.tile_pool(name="pB0", bufs=4, space=bass.MemorySpace.PSUM) as pB0, \
             tc.tile_pool(name="pB1", bufs=4, space=bass.MemorySpace.PSUM) as pB1:
            for hp in range(2):          # heads 2*hp, 2*hp+1
                h0 = 2 * hp
                pb = [[], []]
                for ib in range(nib):
                    pb0_ib = pB0.tile([P, 130], F32, tag="pb0")
                    pb1_ib = pB1.tile([P, 130], F32, tag="pb1")
                    pb[0].append(pb0_ib)
                    pb[1].append(pb1_ib)
                for jt in range(njt):
                    a_row = at_sb[:, jt * rows:(jt + 1) * rows]
                    pan = vfqf[:, jt * PAN_C:(jt + 1) * PAN_C]
                    m2b = []
                    for hh in range(2):
                        h = h0 + hh
                        mb = m_pool.tile([P, rows], F16, tag="mb")
                        nc.vector.tensor_scalar(
                            mb[:], sbc[:, h * rows:(h + 1) * rows],
                            negt[:, jt * 4 + h: jt * 4 + h + 1],
                            None, AL.is_ge)
                        m2 = m2_pool.tile([P, rows], F16, tag="m2b")
                        nc.vector.tensor_tensor(m2[:], mb[:], a_row, AL.mult)
                        m2b.append(m2)
                    for ib in range(nib):
                        for hh in range(2):
                            h = h0 + hh
                            nc.tensor.matmul(
                                pb[hh][ib][:],
                                m2b[hh][:, ib * P:(ib + 1) * P],
                                pan[:, h * VQ_C: h * VQ_C + 2 * GRP].rearrange(
                                    "p (g c) -> p g c", c=GRP)[:, :, 0:65],
                                start=(jt == 0), stop=(jt == njt - 1))
                # epilogue for this head pair
                for ib in range(nib):
                    for hh in range(2):
                        h = h0 + hh
                        dh = e_pool.tile([P, 65], F32, tag="dh")
                        nc.vector.tensor_sub(
                            dh[:], ca_all[ib][:, h * 65:(h + 1) * 65],
                            pb[hh][ib][:, 65:130])
                        zh = e_pool.tile([P, 65], F32, tag="zh")
                        nc.vector.scalar_tensor_tensor(
                            zh[:], pb[hh][ib][:, 0:65],
                            w_cat[:, ib * 4 + h: ib * 4 + h + 1],
                            dh[:], AL.mult, AL.add)
                        rc = e_pool.tile([P, 1], F32, tag="rc")
                        nc.vector.reciprocal(rc[:], zh[:, 64:65])
                        nc.vector.tensor_scalar_mul(
                            out_sbs[ib][:, h * OUT_DIM:(h + 1) * OUT_DIM],
                            zh[:, 0:OUT_DIM], rc[:])
        for ib in range(nib):
            nc.sync.dma_start(OUT[ib * P:(ib + 1) * P, :], out_sbs[ib][:])

    nc.compile()
    return nc


def prep_inputs(X, A, W, a, n_total=N_TOTAL, rows=ROWS, n_cores=N_CORES):
    """Host-side sharding / layout prep.  Returns list of per-core in_maps."""
    f16 = np.float16
    X = np.asarray(X, np.float32)
    A = np.asarray(A)
    W = np.asarray(W, np.float32)
    a = np.asarray(a, np.float32)

    XT = np.ascontiguousarray(X.T).astype(f16)
    Wcat = np.ascontiguousarray(W.transpose(1, 0, 2).reshape(IN_DIM, HEADS * OUT_DIM))
    a_src, a_dst = a[:, :OUT_DIM], a[:, OUT_DIM:]
    w_src = np.einsum('hdo,ho->hd', W, a_src).astype(np.float32)
    w_dst = np.einsum('hdo,ho->hd', W, a_dst).astype(np.float32)
    W8 = np.concatenate([Wcat, w_dst.T], axis=1).astype(f16)
    W4 = np.ascontiguousarray(w_src.T).astype(f16)
    WSRCB = np.repeat(w_src.T[:, :, None], P, axis=2).reshape(IN_DIM, HEADS * P)
    WSRCB = np.ascontiguousarray(WSRCB).astype(f16)

    Af = (A > 0).astype(np.float32)
    in_maps = []
    for c in range(n_cores):
        i0 = c * rows
        at = np.ascontiguousarray(Af[i0:i0 + rows, :].T).astype(f16)
        xtown = np.ascontiguousarray(X[i0:i0 + rows, :].T).astype(f16)
        in_maps.append({
            "XT": XT, "XTOWN": xtown, "W8": W8, "W4": W4,
            "WSRCB": WSRCB, "AT": at,
        })
    return in_maps


_CACHED_NC = None


def _get_nc():
    global _CACHED_NC
    if _CACHED_NC is None:
        _CACHED_NC = build_program()
    return _CACHED_NC


def kernel(X, A, W, a, b, _trace=False, _trace_kwargs=None):
    nc = _get_nc()
    in_maps = prep_inputs(X, A, W, a)
    kw = {}
    if _trace:
        kw["trace"] = True
        if _trace_kwargs:
            kw.update(_trace_kwargs)
    res = run_bass_kernel_spmd(nc, in_maps, core_ids=list(range(N_CORES)), **kw)
    out = np.concatenate([r["OUT"] for r in res.results], axis=0)
    out = out + np.asarray(b, np.float32).reshape(1, HEADS * OUT_DIM)
    if _trace:
        return out.astype(np.float32), res
    return out.astype(np.float32)
